# revision 10
# baseline (speedup 1.0000x reference)
import sys

for _p in ("/opt/trn_rl_repo", "/root/.axon_site"):
    if _p not in sys.path:
        sys.path.insert(0, _p)

from contextlib import ExitStack

import numpy as np
import ml_dtypes

import concourse.bass as bass
import concourse.bacc as bacc
import concourse.mybir as mybir
import concourse.tile as tile
from concourse import bass_utils
from concourse.masks import make_identity

F32 = mybir.dt.float32
F32R = mybir.dt.float32r
BF16 = mybir.dt.bfloat16
I32 = mybir.dt.int32
AF = mybir.ActivationFunctionType

V, E, H = 50000, 256, 256
B, T = 64, 512
G = 3 * H
NCORES = 8
BL = B // NCORES
P = 128
NJ = G // P
W = 2 * BL

_CACHE: dict = {}


def _build(t_steps: int = T):
    n_tok = t_steps * BL
    n_gather = n_tok // P
    chunk_tok = 512 if n_tok >= 512 else n_tok
    n_chunks = n_tok // chunk_tok
    gpc = chunk_tok // P

    nc = bacc.Bacc("TRN2", debug=False, num_devices=NCORES)

    emb = nc.dram_tensor("emb", [V, E], F32, kind="ExternalInput").ap()
    idx = nc.dram_tensor("idx", [P, n_gather], I32, kind="ExternalInput").ap()
    wih_t = nc.dram_tensor("wih_t", [E, G], BF16, kind="ExternalInput").ap()
    whh_t = nc.dram_tensor("whh_t", [E, G], BF16, kind="ExternalInput").ap()
    xbias = nc.dram_tensor("xbias", [P, NJ], F32, kind="ExternalInput").ap()
    bn = nc.dram_tensor("bn", [P, W], BF16, kind="ExternalInput").ap()
    out_d = nc.dram_tensor("out", [P, t_steps * W], BF16, kind="ExternalOutput").ap()

    with tile.TileContext(nc) as tc, ExitStack() as ctx:
        const = ctx.enter_context(tc.tile_pool(name="const", bufs=1))
        big = ctx.enter_context(tc.tile_pool(name="big", bufs=1))

        ident = const.tile([P, P], F32)
        make_identity(nc, ident[:])
        idx_sb = const.tile([P, n_gather], I32)
        nc.sync.dma_start(idx_sb[:], idx[:])
        wih_sb = [const.tile([P, G], BF16, name=f"wih{k}") for k in range(2)]
        whh_sb = [const.tile([P, G], BF16, name=f"whh{k}") for k in range(2)]
        for k in range(2):
            nc.sync.dma_start(wih_sb[k][:], wih_t[P * k : P * (k + 1), :])
            nc.sync.dma_start(whh_sb[k][:], whh_t[P * k : P * (k + 1), :])
        xbias_sb = const.tile([P, NJ], F32)
        nc.sync.dma_start(xbias_sb[:], xbias[:])
        bn_sb = const.tile([P, W], BF16)
        nc.sync.dma_start(bn_sb[:], bn[:])
        identb = const.tile([P, P], BF16)
        make_identity(nc, identb[:])
        h0 = const.tile([P, W], BF16)
        nc.vector.memset(h0[:], 0.0)

        xp_sb = big.tile([P, t_steps, 6 * BL], BF16)
        out_sb = big.tile([P, t_steps, W], BF16)

        gat = ctx.enter_context(tc.tile_pool(name="gat", bufs=3))
        embt = ctx.enter_context(tc.tile_pool(name="embt", bufs=4))
        tpps = ctx.enter_context(tc.tile_pool(name="tpps", bufs=2, space="PSUM"))
        xpps = ctx.enter_context(tc.tile_pool(name="xpps", bufs=2, space="PSUM"))

        chunks = []
        pos = 0
        while pos < n_tok:
            size = 128 if pos < 512 else 512
            size = min(size, n_tok - pos)
            chunks.append((pos, size))
            pos += size

        def chunk_thunks(ci, start, size):
            gpc = size // P
            et = [
                embt.tile([P, size], BF16, tag=f"et{k}", name=f"et{k}_{ci}")
                for k in range(2)
            ]

            def gather(g):
                def f():
                    gt = gat.tile([P, E], F32, tag="gt", name=f"gt_{ci}_{g}")
                    nc.gpsimd.indirect_dma_start(
                        out=gt[:],
                        out_offset=None,
                        in_=emb[:],
                        in_offset=bass.IndirectOffsetOnAxis(
                            ap=idx_sb[:, start // P + g : start // P + g + 1], axis=0
                        ),
                    )
                    return gt
                return f

            gt_holder = {}

            def do_gather(g):
                def f():
                    gt_holder[g] = gather(g)()
                return f

            def transpose_copy(g, k):
                def f():
                    gt = gt_holder[g]
                    tp = tpps.tile([P, P], F32, tag="tp", name=f"tp_{ci}_{g}_{k}")
                    nc.tensor.transpose(tp[:], gt[:, P * k : P * (k + 1)], ident[:])
                    dst = et[k][:, P * g : P * (g + 1)]
                    if (g * 2 + k) % 2 == 0:
                        nc.vector.tensor_copy(dst, tp[:])
                    else:
                        nc.scalar.copy(dst, tp[:])
                return f

            def gemm_repack(j):
                def f():
                    xpp = xpps.tile([P, size], F32, tag="xpp", name=f"xpp_{ci}_{j}")
                    for k in range(2):
                        nc.tensor.matmul(
                            xpp[:],
                            lhsT=wih_sb[k][:, P * j : P * (j + 1)],
                            rhs=et[k][:],
                            start=(k == 0),
                            stop=(k == 1),
                        )
                    tpc = size // BL
                    t0 = start // BL
                    dst = xp_sb[:, t0 : t0 + tpc, BL * j : BL * (j + 1)]
                    src = xpp[:].rearrange("p (t b) -> p t b", b=BL)
                    if j % 2 == 0:
                        nc.vector.tensor_scalar_add(dst, src, xbias_sb[:, j : j + 1])
                    else:
                        nc.scalar.add(dst, src, xbias_sb[:, j : j + 1])
                return f

            out = []
            for g in range(gpc):
                out.append(do_gather(g))
                for k in range(2):
                    out.append(transpose_copy(g, k))
            for j in range(NJ):
                out.append(gemm_repack(j))
            return out

        pending = []
        for ci, (start, size) in enumerate(chunks):
            th = chunk_thunks(ci, start, size)
            if ci == 0:
                for f in th:
                    f()
            else:
                pending.extend(th)
        pending.reverse()

        with (
            tc.tile_pool(name="rzps", bufs=2, space="PSUM") as rzps,
            tc.tile_pool(name="nps", bufs=2, space="PSUM") as nps,
            tc.tile_pool(name="gates", bufs=3) as gp,
        ):
            h_prev = h0
            for t in range(t_steps):
                n_drip = 2 if t < 48 else 1
                for _ in range(n_drip):
                    if pending:
                        pending.pop()()
                rzp = rzps.tile([P, 4 * BL], F32)
                npp = nps.tile([P, W], F32)
                rhs = [h_prev[:, BL * k : BL * (k + 1)] for k in range(2)]
                for j in range(4):
                    nc.tensor.matmul(
                        rzp[:, BL * j : BL * (j + 1)],
                        lhsT=identb[:],
                        rhs=xp_sb[:, t, BL * j : BL * (j + 1)],
                        start=(j == 0),
                        stop=False,
                        skip_group_check=True,
                    )
                for j in range(2):
                    nc.tensor.matmul(
                        npp[:, BL * j : BL * (j + 1)],
                        lhsT=identb[:],
                        rhs=bn_sb[:, BL * j : BL * (j + 1)],
                        start=(j == 0),
                        stop=False,
                        skip_group_check=True,
                    )
                for j in range(4):
                    for k in range(2):
                        nc.tensor.matmul(
                            rzp[:, BL * j : BL * (j + 1)],
                            lhsT=whh_sb[k][:, P * j : P * (j + 1)],
                            rhs=rhs[k],
                            start=False,
                            stop=(j == 3 and k == 1),
                            skip_group_check=True,
                        )
                for j in range(2):
                    for k in range(2):
                        nc.tensor.matmul(
                            npp[:, BL * j : BL * (j + 1)],
                            lhsT=whh_sb[k][:, P * (j + 4) : P * (j + 5)],
                            rhs=rhs[k],
                            start=False,
                            stop=(j == 1 and k == 1),
                            skip_group_check=True,
                        )
                rz = gp.tile([P, 4 * BL], F32, tag="rz")
                nc.scalar.activation(rz[:], rzp[:], AF.Sigmoid)
                zc = gp.tile([P, W], F32, tag="zc")
                nc.scalar.activation(zc[:], rzp[:, 2 * BL : 4 * BL], AF.Sigmoid, scale=-1.0)
                u = gp.tile([P, W], F32, tag="u")
                nc.vector.tensor_mul(u[:], rz[:, 0 : 2 * BL], npp[:])
                v = gp.tile([P, W], F32, tag="v")
                nc.vector.tensor_add(v[:], u[:], xp_sb[:, t, 4 * BL : 6 * BL])
                n_ = gp.tile([P, W], F32, tag="n")
                nc.scalar.activation(n_[:], v[:], AF.Tanh)
                c1 = gp.tile([P, W], F32, tag="c1")
                nc.vector.tensor_mul(c1[:], rz[:, 2 * BL : 4 * BL], h_prev[:])
                c3 = gp.tile([P, W], F32, tag="c3")
                nc.vector.tensor_mul(c3[:], zc[:], n_[:])
                h_new = out_sb[:, t, :]
                nc.vector.tensor_add(h_new, c3[:], c1[:])
                h_prev = h_new

        nc.sync.dma_start(out_d[:], out_sb[:].rearrange("p t w -> p (t w)"))

    nc.compile()
    return nc


def _prep_shared(embedding, W_ih, W_hh, b_ih, b_hh):
    emb = np.ascontiguousarray(np.asarray(embedding, np.float32))
    wih_t = np.ascontiguousarray(np.asarray(W_ih, np.float32).T.astype(ml_dtypes.bfloat16))
    whh_t = np.ascontiguousarray(np.asarray(W_hh, np.float32).T.astype(ml_dtypes.bfloat16))
    b_ih = np.asarray(b_ih, np.float32)
    b_hh = np.asarray(b_hh, np.float32)
    bias_x = b_ih.copy()
    bias_x[: 2 * H] += b_hh[: 2 * H]
    xbias = np.ascontiguousarray(bias_x.reshape(NJ, P).T)
    bn = np.ascontiguousarray(
        np.broadcast_to(b_hh[2 * H :].reshape(2, P).T[:, :, None], (P, 2, BL))
    ).reshape(P, W).astype(ml_dtypes.bfloat16)
    return emb, wih_t, whh_t, xbias, bn


def _get_nc_and_inmaps(input, embedding, W_ih, W_hh, b_ih, b_hh, ts):
    input = np.asarray(input)
    if "nc" not in _CACHE or _CACHE.get("ts") != ts:
        _CACHE["nc"] = _build(ts)
        _CACHE["ts"] = ts
    nc = _CACHE["nc"]

    emb, wih_t, whh_t, xbias, bn = _prep_shared(embedding, W_ih, W_hh, b_ih, b_hh)

    in_maps = []
    for c in range(NCORES):
        ids = np.asarray(input[c * BL : (c + 1) * BL, :ts], np.int32)
        idx = np.ascontiguousarray(ids.T.reshape(-1).reshape(ts * BL // P, P).T)
        in_maps.append(
            {
                "emb": emb,
                "idx": idx,
                "wih_t": wih_t,
                "whh_t": whh_t,
                "xbias": xbias,
                "bn": bn,
            }
        )
    return nc, in_maps


def run_traced(input, embedding, W_ih, W_hh, b_ih, b_hh, _t_steps: int = T):
    nc, in_maps = _get_nc_and_inmaps(input, embedding, W_ih, W_hh, b_ih, b_hh, _t_steps)
    return bass_utils.run_bass_kernel_spmd(
        nc, in_maps, core_ids=list(range(NCORES)), trace=True, trace_cores=[0]
    )


def kernel(input, embedding, W_ih, W_hh, b_ih, b_hh, _t_steps: int = T):
    ts = _t_steps
    nc, in_maps = _get_nc_and_inmaps(input, embedding, W_ih, W_hh, b_ih, b_hh, ts)

    res = bass_utils.run_bass_kernel_spmd(nc, in_maps, core_ids=list(range(NCORES)))

    outputs = np.empty((ts, B, H), np.float32)
    for c in range(NCORES):
        buf = np.asarray(res.results[c]["out"], np.float32).reshape(P, ts, 2, BL)
        outputs[:, c * BL : (c + 1) * BL, :] = (
            buf.transpose(1, 3, 2, 0).reshape(ts, BL, H)
        )
    hidden = outputs[-1][None]
    return outputs, hidden


if __name__ == "__main__":
    rng = np.random.default_rng(0)
    ts = int(sys.argv[1]) if len(sys.argv) > 1 else 32
    inp = rng.integers(0, V, (B, T)).astype(np.int32)
    emb = rng.uniform(-0.05, 0.05, (V, E)).astype(np.float32)
    emb[0] = 0
    wih = rng.uniform(-0.05, 0.05, (G, E)).astype(np.float32)
    whh = rng.uniform(-0.05, 0.05, (G, H)).astype(np.float32)
    bih = rng.uniform(-0.05, 0.05, G).astype(np.float32)
    bhh = rng.uniform(-0.05, 0.05, G).astype(np.float32)

    outs, hid = kernel(inp, emb, wih, whh, bih, bhh, _t_steps=ts)

    x = emb[inp[:, :ts]].transpose(1, 0, 2)
    xp = x @ wih.T + bih
    h = np.zeros((B, H), np.float32)
    sig = lambda a: 1.0 / (1.0 + np.exp(-a))
    ref = np.empty((ts, B, H), np.float32)
    for t in range(ts):
        hp = h @ whh.T + bhh
        xr, xz, xn = np.split(xp[t], 3, -1)
        hr, hz, hn = np.split(hp, 3, -1)
        r = sig(xr + hr)
        z = sig(xz + hz)
        n = np.tanh(xn + r * hn)
        h = (1 - z) * n + z * h
        ref[t] = h
    err = np.abs(outs - ref).max() / (np.abs(ref).max() + 1e-12)
    print("max err vs absmax:", err)
    print("sample:", outs[0, 0, :4], ref[0, 0, :4])


# revision 11
# speedup vs baseline: 1.1854x; 1.1854x over previous
import sys

for _p in ("/opt/trn_rl_repo", "/root/.axon_site"):
    if _p not in sys.path:
        sys.path.insert(0, _p)

from contextlib import ExitStack

import numpy as np
import ml_dtypes

import concourse.bass as bass
import concourse.bacc as bacc
import concourse.mybir as mybir
import concourse.tile as tile
from concourse import bass_utils
from concourse.masks import make_identity

F32 = mybir.dt.float32
F32R = mybir.dt.float32r
BF16 = mybir.dt.bfloat16
I32 = mybir.dt.int32
AF = mybir.ActivationFunctionType

V, E, H = 50000, 256, 256
B, T = 64, 512
G = 3 * H
NCORES = 8
BL = B // NCORES
P = 128
NJ = G // P
W = 2 * BL

_CACHE: dict = {}


def _build(t_steps: int = T):
    n_tok = t_steps * BL
    n_gather = n_tok // P
    chunk_tok = 512 if n_tok >= 512 else n_tok
    n_chunks = n_tok // chunk_tok
    gpc = chunk_tok // P

    nc = bacc.Bacc("TRN2", debug=False, num_devices=NCORES)

    emb = nc.dram_tensor("emb", [V, E], F32, kind="ExternalInput").ap()
    idx = nc.dram_tensor("idx", [P, n_gather], I32, kind="ExternalInput").ap()
    wih_t = nc.dram_tensor("wih_t", [E, G], BF16, kind="ExternalInput").ap()
    whh_t = nc.dram_tensor("whh_t", [E, G], BF16, kind="ExternalInput").ap()
    xbias = nc.dram_tensor("xbias", [P, NJ], F32, kind="ExternalInput").ap()
    bn = nc.dram_tensor("bn", [P, W], BF16, kind="ExternalInput").ap()
    out_d = nc.dram_tensor("out", [P, t_steps * W], BF16, kind="ExternalOutput").ap()

    with tile.TileContext(nc) as tc, ExitStack() as ctx:
        const = ctx.enter_context(tc.tile_pool(name="const", bufs=1))
        big = ctx.enter_context(tc.tile_pool(name="big", bufs=1))

        ident = const.tile([P, P], F32)
        make_identity(nc, ident[:])
        idx_sb = const.tile([P, n_gather], I32)
        nc.sync.dma_start(idx_sb[:], idx[:])
        wih_sb = [const.tile([P, G], BF16, name=f"wih{k}") for k in range(2)]
        whh_sb = [const.tile([P, G], BF16, name=f"whh{k}") for k in range(2)]
        for k in range(2):
            nc.sync.dma_start(wih_sb[k][:], wih_t[P * k : P * (k + 1), :])
            nc.sync.dma_start(whh_sb[k][:], whh_t[P * k : P * (k + 1), :])
        xbias_sb = const.tile([P, NJ], F32)
        nc.sync.dma_start(xbias_sb[:], xbias[:])
        bn_sb = const.tile([P, W], BF16)
        nc.sync.dma_start(bn_sb[:], bn[:])
        identb = const.tile([P, P], BF16)
        make_identity(nc, identb[:])
        h0 = const.tile([P, W], BF16)
        nc.vector.memset(h0[:], 0.0)

        xp_sb = big.tile([P, t_steps, 6 * BL], BF16)
        out_sb = big.tile([P, t_steps, W], BF16)

        gat = ctx.enter_context(tc.tile_pool(name="gat", bufs=3))
        embt = ctx.enter_context(tc.tile_pool(name="embt", bufs=4))
        tpps = ctx.enter_context(tc.tile_pool(name="tpps", bufs=2, space="PSUM"))
        xpps = ctx.enter_context(tc.tile_pool(name="xpps", bufs=2, space="PSUM"))

        chunks = []
        pos = 0
        while pos < n_tok:
            size = 128 if pos < 512 else 512
            size = min(size, n_tok - pos)
            chunks.append((pos, size))
            pos += size

        def chunk_thunks(ci, start, size):
            gpc = size // P
            et = [
                embt.tile([P, size], BF16, tag=f"et{k}", name=f"et{k}_{ci}")
                for k in range(2)
            ]

            def gather(g):
                def f():
                    gt = gat.tile([P, E], F32, tag="gt", name=f"gt_{ci}_{g}")
                    nc.gpsimd.indirect_dma_start(
                        out=gt[:],
                        out_offset=None,
                        in_=emb[:],
                        in_offset=bass.IndirectOffsetOnAxis(
                            ap=idx_sb[:, start // P + g : start // P + g + 1], axis=0
                        ),
                    )
                    return gt
                return f

            gt_holder = {}

            def do_gather(g):
                def f():
                    gt_holder[g] = gather(g)()
                return f

            def transpose_copy(g, k):
                def f():
                    gt = gt_holder[g]
                    tp = tpps.tile([P, P], F32, tag="tp", name=f"tp_{ci}_{g}_{k}")
                    nc.tensor.transpose(tp[:], gt[:, P * k : P * (k + 1)], ident[:])
                    dst = et[k][:, P * g : P * (g + 1)]
                    if (g * 2 + k) % 2 == 0:
                        nc.vector.tensor_copy(dst, tp[:])
                    else:
                        nc.scalar.copy(dst, tp[:])
                return f

            def gemm_repack(j):
                def f():
                    xpp = xpps.tile([P, size], F32, tag="xpp", name=f"xpp_{ci}_{j}")
                    for k in range(2):
                        nc.tensor.matmul(
                            xpp[:],
                            lhsT=wih_sb[k][:, P * j : P * (j + 1)],
                            rhs=et[k][:],
                            start=(k == 0),
                            stop=(k == 1),
                        )
                    tpc = size // BL
                    t0 = start // BL
                    dst = xp_sb[:, t0 : t0 + tpc, BL * j : BL * (j + 1)]
                    src = xpp[:].rearrange("p (t b) -> p t b", b=BL)
                    if j % 2 == 0:
                        nc.vector.tensor_scalar_add(dst, src, xbias_sb[:, j : j + 1])
                    else:
                        nc.scalar.add(dst, src, xbias_sb[:, j : j + 1])
                return f

            out = []
            for g in range(gpc):
                out.append(do_gather(g))
                for k in range(2):
                    out.append(transpose_copy(g, k))
            for j in range(NJ):
                out.append(gemm_repack(j))
            return out

        for ci, (start, size) in enumerate(chunks):
            for f in chunk_thunks(ci, start, size):
                f()

        with (
            tc.tile_pool(name="rzps", bufs=2, space="PSUM") as rzps,
            tc.tile_pool(name="nps", bufs=2, space="PSUM") as nps,
            tc.tile_pool(name="gates", bufs=3) as gp,
        ):
            h_prev = h0
            for t in range(t_steps):
                rzp = rzps.tile([P, 4 * BL], F32)
                npp = nps.tile([P, W], F32)
                rhs = [h_prev[:, BL * k : BL * (k + 1)] for k in range(2)]
                for j in range(4):
                    nc.tensor.matmul(
                        rzp[:, BL * j : BL * (j + 1)],
                        lhsT=identb[:],
                        rhs=xp_sb[:, t, BL * j : BL * (j + 1)],
                        start=(j == 0),
                        stop=False,
                        skip_group_check=True,
                    )
                for j in range(2):
                    nc.tensor.matmul(
                        npp[:, BL * j : BL * (j + 1)],
                        lhsT=identb[:],
                        rhs=bn_sb[:, BL * j : BL * (j + 1)],
                        start=(j == 0),
                        stop=False,
                        skip_group_check=True,
                    )
                for j in range(4):
                    for k in range(2):
                        nc.tensor.matmul(
                            rzp[:, BL * j : BL * (j + 1)],
                            lhsT=whh_sb[k][:, P * j : P * (j + 1)],
                            rhs=rhs[k],
                            start=False,
                            stop=(j == 3 and k == 1),
                            skip_group_check=True,
                        )
                for j in range(2):
                    for k in range(2):
                        nc.tensor.matmul(
                            npp[:, BL * j : BL * (j + 1)],
                            lhsT=whh_sb[k][:, P * (j + 4) : P * (j + 5)],
                            rhs=rhs[k],
                            start=False,
                            stop=(j == 1 and k == 1),
                            skip_group_check=True,
                        )
                rz = gp.tile([P, 4 * BL], F32, tag="rz")
                nc.scalar.activation(rz[:], rzp[:], AF.Sigmoid)
                zc = gp.tile([P, W], F32, tag="zc")
                nc.scalar.activation(zc[:], rzp[:, 2 * BL : 4 * BL], AF.Sigmoid, scale=-1.0)
                u = gp.tile([P, W], F32, tag="u")
                nc.vector.tensor_mul(u[:], rz[:, 0 : 2 * BL], npp[:])
                v = gp.tile([P, W], F32, tag="v")
                nc.vector.tensor_add(v[:], u[:], xp_sb[:, t, 4 * BL : 6 * BL])
                n_ = gp.tile([P, W], F32, tag="n")
                nc.scalar.activation(n_[:], v[:], AF.Tanh)
                c1 = gp.tile([P, W], F32, tag="c1")
                nc.vector.tensor_mul(c1[:], rz[:, 2 * BL : 4 * BL], h_prev[:])
                c3 = gp.tile([P, W], F32, tag="c3")
                nc.vector.tensor_mul(c3[:], zc[:], n_[:])
                h_new = out_sb[:, t, :]
                nc.vector.tensor_add(h_new, c3[:], c1[:])
                h_prev = h_new

        nc.sync.dma_start(out_d[:], out_sb[:].rearrange("p t w -> p (t w)"))

    nc.compile()
    return nc


def _prep_shared(embedding, W_ih, W_hh, b_ih, b_hh):
    emb = np.ascontiguousarray(np.asarray(embedding, np.float32))
    wih_t = np.ascontiguousarray(np.asarray(W_ih, np.float32).T.astype(ml_dtypes.bfloat16))
    whh_t = np.ascontiguousarray(np.asarray(W_hh, np.float32).T.astype(ml_dtypes.bfloat16))
    b_ih = np.asarray(b_ih, np.float32)
    b_hh = np.asarray(b_hh, np.float32)
    bias_x = b_ih.copy()
    bias_x[: 2 * H] += b_hh[: 2 * H]
    xbias = np.ascontiguousarray(bias_x.reshape(NJ, P).T)
    bn = np.ascontiguousarray(
        np.broadcast_to(b_hh[2 * H :].reshape(2, P).T[:, :, None], (P, 2, BL))
    ).reshape(P, W).astype(ml_dtypes.bfloat16)
    return emb, wih_t, whh_t, xbias, bn


def _get_nc_and_inmaps(input, embedding, W_ih, W_hh, b_ih, b_hh, ts):
    input = np.asarray(input)
    if "nc" not in _CACHE or _CACHE.get("ts") != ts:
        _CACHE["nc"] = _build(ts)
        _CACHE["ts"] = ts
    nc = _CACHE["nc"]

    emb, wih_t, whh_t, xbias, bn = _prep_shared(embedding, W_ih, W_hh, b_ih, b_hh)

    in_maps = []
    for c in range(NCORES):
        ids = np.asarray(input[c * BL : (c + 1) * BL, :ts], np.int32)
        idx = np.ascontiguousarray(ids.T.reshape(-1).reshape(ts * BL // P, P).T)
        in_maps.append(
            {
                "emb": emb,
                "idx": idx,
                "wih_t": wih_t,
                "whh_t": whh_t,
                "xbias": xbias,
                "bn": bn,
            }
        )
    return nc, in_maps


def run_traced(input, embedding, W_ih, W_hh, b_ih, b_hh, _t_steps: int = T):
    nc, in_maps = _get_nc_and_inmaps(input, embedding, W_ih, W_hh, b_ih, b_hh, _t_steps)
    return bass_utils.run_bass_kernel_spmd(
        nc, in_maps, core_ids=list(range(NCORES)), trace=True, trace_cores=[0]
    )


def kernel(input, embedding, W_ih, W_hh, b_ih, b_hh, _t_steps: int = T):
    ts = _t_steps
    nc, in_maps = _get_nc_and_inmaps(input, embedding, W_ih, W_hh, b_ih, b_hh, ts)

    res = bass_utils.run_bass_kernel_spmd(nc, in_maps, core_ids=list(range(NCORES)))

    outputs = np.empty((ts, B, H), np.float32)
    for c in range(NCORES):
        buf = np.asarray(res.results[c]["out"], np.float32).reshape(P, ts, 2, BL)
        outputs[:, c * BL : (c + 1) * BL, :] = (
            buf.transpose(1, 3, 2, 0).reshape(ts, BL, H)
        )
    hidden = outputs[-1][None]
    return outputs, hidden


if __name__ == "__main__":
    rng = np.random.default_rng(0)
    ts = int(sys.argv[1]) if len(sys.argv) > 1 else 32
    inp = rng.integers(0, V, (B, T)).astype(np.int32)
    emb = rng.uniform(-0.05, 0.05, (V, E)).astype(np.float32)
    emb[0] = 0
    wih = rng.uniform(-0.05, 0.05, (G, E)).astype(np.float32)
    whh = rng.uniform(-0.05, 0.05, (G, H)).astype(np.float32)
    bih = rng.uniform(-0.05, 0.05, G).astype(np.float32)
    bhh = rng.uniform(-0.05, 0.05, G).astype(np.float32)

    outs, hid = kernel(inp, emb, wih, whh, bih, bhh, _t_steps=ts)

    x = emb[inp[:, :ts]].transpose(1, 0, 2)
    xp = x @ wih.T + bih
    h = np.zeros((B, H), np.float32)
    sig = lambda a: 1.0 / (1.0 + np.exp(-a))
    ref = np.empty((ts, B, H), np.float32)
    for t in range(ts):
        hp = h @ whh.T + bhh
        xr, xz, xn = np.split(xp[t], 3, -1)
        hr, hz, hn = np.split(hp, 3, -1)
        r = sig(xr + hr)
        z = sig(xz + hz)
        n = np.tanh(xn + r * hn)
        h = (1 - z) * n + z * h
        ref[t] = h
    err = np.abs(outs - ref).max() / (np.abs(ref).max() + 1e-12)
    print("max err vs absmax:", err)
    print("sample:", outs[0, 0, :4], ref[0, 0, :4])


# revision 12
# speedup vs baseline: 1.3188x; 1.1126x over previous
import sys

for _p in ("/opt/trn_rl_repo", "/root/.axon_site"):
    if _p not in sys.path:
        sys.path.insert(0, _p)

from contextlib import ExitStack

import numpy as np
import ml_dtypes

import concourse.bass as bass
import concourse.bacc as bacc
import concourse.mybir as mybir
import concourse.tile as tile
from concourse import bass_utils
from concourse.masks import make_identity

F32 = mybir.dt.float32
F32R = mybir.dt.float32r
BF16 = mybir.dt.bfloat16
I32 = mybir.dt.int32
AF = mybir.ActivationFunctionType

V, E, H = 50000, 256, 256
B, T = 64, 512
G = 3 * H
NCORES = 8
BL = B // NCORES
P = 128
NJ = G // P
W = 2 * BL

_CACHE: dict = {}


def _build(t_steps: int = T):
    n_tok = t_steps * BL
    n_gather = n_tok // P
    chunk_tok = 512 if n_tok >= 512 else n_tok
    n_chunks = n_tok // chunk_tok
    gpc = chunk_tok // P

    nc = bacc.Bacc("TRN2", debug=False, num_devices=NCORES)

    emb = nc.dram_tensor("emb", [V, E], F32, kind="ExternalInput").ap()
    idx = nc.dram_tensor("idx", [P, n_gather], I32, kind="ExternalInput").ap()
    wih_t = nc.dram_tensor("wih_t", [E, G], BF16, kind="ExternalInput").ap()
    whh_t = nc.dram_tensor("whh_t", [E, G], BF16, kind="ExternalInput").ap()
    xbias = nc.dram_tensor("xbias", [P, NJ], F32, kind="ExternalInput").ap()
    bn = nc.dram_tensor("bn", [P, W], BF16, kind="ExternalInput").ap()
    out_d = nc.dram_tensor("out", [P, t_steps * W], BF16, kind="ExternalOutput").ap()

    with tile.TileContext(nc) as tc, ExitStack() as ctx:
        const = ctx.enter_context(tc.tile_pool(name="const", bufs=1))
        big = ctx.enter_context(tc.tile_pool(name="big", bufs=1))

        ident = const.tile([P, P], F32)
        make_identity(nc, ident[:])
        idx_sb = const.tile([P, n_gather], I32)
        nc.sync.dma_start(idx_sb[:], idx[:])
        wih_sb = [const.tile([P, G], BF16, name=f"wih{k}") for k in range(2)]
        whh_sb = [const.tile([P, G], BF16, name=f"whh{k}") for k in range(2)]
        for k in range(2):
            nc.sync.dma_start(wih_sb[k][:], wih_t[P * k : P * (k + 1), :])
            nc.sync.dma_start(whh_sb[k][:], whh_t[P * k : P * (k + 1), :])
        xbias_sb = const.tile([P, NJ], F32)
        nc.sync.dma_start(xbias_sb[:], xbias[:])
        bn_sb = const.tile([P, W], BF16)
        nc.sync.dma_start(bn_sb[:], bn[:])
        identb = const.tile([P, P], BF16)
        make_identity(nc, identb[:])
        h0 = const.tile([P, W], BF16)
        nc.vector.memset(h0[:], 0.0)

        xp_sb = big.tile([P, t_steps, 6 * BL], BF16)
        out_sb = big.tile([P, t_steps, W], BF16)

        gat = ctx.enter_context(tc.tile_pool(name="gat", bufs=3))
        embt = ctx.enter_context(tc.tile_pool(name="embt", bufs=4))
        tpps = ctx.enter_context(tc.tile_pool(name="tpps", bufs=2, space="PSUM"))
        xpps = ctx.enter_context(tc.tile_pool(name="xpps", bufs=2, space="PSUM"))

        chunks = []
        pos = 0
        while pos < n_tok:
            size = 128 if pos < 512 else 512
            size = min(size, n_tok - pos)
            chunks.append((pos, size))
            pos += size

        def chunk_thunks(ci, start, size):
            gpc = size // P
            et = [
                embt.tile([P, size], BF16, tag=f"et{k}", name=f"et{k}_{ci}")
                for k in range(2)
            ]

            def gather(g):
                def f():
                    gt = gat.tile([P, E], F32, tag="gt", name=f"gt_{ci}_{g}")
                    nc.gpsimd.indirect_dma_start(
                        out=gt[:],
                        out_offset=None,
                        in_=emb[:],
                        in_offset=bass.IndirectOffsetOnAxis(
                            ap=idx_sb[:, start // P + g : start // P + g + 1], axis=0
                        ),
                    )
                    return gt
                return f

            gt_holder = {}

            def do_gather(g):
                def f():
                    gt_holder[g] = gather(g)()
                return f

            def transpose_copy(g, k):
                def f():
                    gt = gt_holder[g]
                    tp = tpps.tile([P, P], F32, tag="tp", name=f"tp_{ci}_{g}_{k}")
                    nc.tensor.transpose(tp[:], gt[:, P * k : P * (k + 1)], ident[:])
                    dst = et[k][:, P * g : P * (g + 1)]
                    if (g * 2 + k) % 2 == 0:
                        nc.vector.tensor_copy(dst, tp[:])
                    else:
                        nc.scalar.copy(dst, tp[:])
                return f

            def gemm_repack(j):
                def f():
                    xpp = xpps.tile([P, size], F32, tag="xpp", name=f"xpp_{ci}_{j}")
                    for k in range(2):
                        nc.tensor.matmul(
                            xpp[:],
                            lhsT=wih_sb[k][:, P * j : P * (j + 1)],
                            rhs=et[k][:],
                            start=(k == 0),
                            stop=(k == 1),
                        )
                    tpc = size // BL
                    t0 = start // BL
                    dst = xp_sb[:, t0 : t0 + tpc, BL * j : BL * (j + 1)]
                    src = xpp[:].rearrange("p (t b) -> p t b", b=BL)
                    if j % 2 == 0:
                        nc.vector.tensor_scalar_add(dst, src, xbias_sb[:, j : j + 1])
                    else:
                        nc.scalar.add(dst, src, xbias_sb[:, j : j + 1])
                return f

            out = []
            for g in range(gpc):
                out.append(do_gather(g))
                for k in range(2):
                    out.append(transpose_copy(g, k))
            for j in range(NJ):
                out.append(gemm_repack(j))
            return out

        for ci, (start, size) in enumerate(chunks):
            for f in chunk_thunks(ci, start, size):
                f()

        with (
            tc.tile_pool(name="rzps", bufs=2, space="PSUM") as rzps,
            tc.tile_pool(name="nps", bufs=2, space="PSUM") as nps,
            tc.tile_pool(name="gates", bufs=3) as gp,
        ):
            def inject_const(rzp, npp, t, with_stop):
                for j in range(4):
                    nc.tensor.matmul(
                        rzp[:, BL * j : BL * (j + 1)],
                        lhsT=identb[:],
                        rhs=xp_sb[:, t, BL * j : BL * (j + 1)],
                        start=(j == 0),
                        stop=(with_stop and j == 3),
                        skip_group_check=True,
                    )
                for j in range(2):
                    nc.tensor.matmul(
                        npp[:, BL * j : BL * (j + 1)],
                        lhsT=identb[:],
                        rhs=bn_sb[:, BL * j : BL * (j + 1)],
                        start=(j == 0),
                        stop=(with_stop and j == 1),
                        skip_group_check=True,
                    )

            def w_mms(rzp, npp, cpart, with_stop):
                rhs = [cpart[:, BL * k : BL * (k + 1)] for k in range(2)]
                for j in range(4):
                    for k in range(2):
                        nc.tensor.matmul(
                            rzp[:, BL * j : BL * (j + 1)],
                            lhsT=whh_sb[k][:, P * j : P * (j + 1)],
                            rhs=rhs[k],
                            start=False,
                            stop=(with_stop and j == 3 and k == 1),
                            skip_group_check=True,
                        )
                for j in range(2):
                    for k in range(2):
                        nc.tensor.matmul(
                            npp[:, BL * j : BL * (j + 1)],
                            lhsT=whh_sb[k][:, P * (j + 4) : P * (j + 5)],
                            rhs=rhs[k],
                            start=False,
                            stop=(with_stop and j == 1 and k == 1),
                            skip_group_check=True,
                        )

            h_prev = h0
            rzp_t = rzps.tile([P, 4 * BL], F32, tag="rzp", name="rzp_0")
            npp_t = nps.tile([P, W], F32, tag="npp", name="npp_0")
            inject_const(rzp_t, npp_t, 0, with_stop=True)
            for t in range(t_steps):
                last = t + 1 >= t_steps
                if not last:
                    rzp_n = rzps.tile([P, 4 * BL], F32, tag="rzp", name=f"rzp_{t+1}")
                    npp_n = nps.tile([P, W], F32, tag="npp", name=f"npp_{t+1}")
                    inject_const(rzp_n, npp_n, t + 1, with_stop=False)
                rz = gp.tile([P, 4 * BL], F32, tag="rz")
                nc.scalar.activation(rz[:], rzp_t[:], AF.Sigmoid)
                zc = gp.tile([P, W], F32, tag="zc")
                nc.scalar.activation(zc[:], rzp_t[:, 2 * BL : 4 * BL], AF.Sigmoid, scale=-1.0)
                u = gp.tile([P, W], F32, tag="u")
                nc.vector.tensor_mul(u[:], rz[:, 0 : 2 * BL], npp_t[:])
                v = gp.tile([P, W], F32, tag="v")
                nc.vector.tensor_add(v[:], u[:], xp_sb[:, t, 4 * BL : 6 * BL])
                c1 = gp.tile([P, W], BF16, tag="c1")
                nc.vector.tensor_mul(c1[:], rz[:, 2 * BL : 4 * BL], h_prev[:])
                if not last:
                    w_mms(rzp_n, npp_n, c1, with_stop=False)
                n_ = gp.tile([P, W], F32, tag="n")
                nc.scalar.activation(n_[:], v[:], AF.Tanh)
                c3 = gp.tile([P, W], BF16, tag="c3")
                nc.vector.tensor_mul(c3[:], zc[:], n_[:])
                if not last:
                    w_mms(rzp_n, npp_n, c3, with_stop=True)
                h_new = out_sb[:, t, :]
                nc.vector.tensor_add(h_new, c3[:], c1[:])
                h_prev = h_new
                if not last:
                    rzp_t, npp_t = rzp_n, npp_n

        nc.sync.dma_start(out_d[:], out_sb[:].rearrange("p t w -> p (t w)"))

    nc.compile()
    return nc


def _prep_shared(embedding, W_ih, W_hh, b_ih, b_hh):
    emb = np.ascontiguousarray(np.asarray(embedding, np.float32))
    wih_t = np.ascontiguousarray(np.asarray(W_ih, np.float32).T.astype(ml_dtypes.bfloat16))
    whh_t = np.ascontiguousarray(np.asarray(W_hh, np.float32).T.astype(ml_dtypes.bfloat16))
    b_ih = np.asarray(b_ih, np.float32)
    b_hh = np.asarray(b_hh, np.float32)
    bias_x = b_ih.copy()
    bias_x[: 2 * H] += b_hh[: 2 * H]
    xbias = np.ascontiguousarray(bias_x.reshape(NJ, P).T)
    bn = np.ascontiguousarray(
        np.broadcast_to(b_hh[2 * H :].reshape(2, P).T[:, :, None], (P, 2, BL))
    ).reshape(P, W).astype(ml_dtypes.bfloat16)
    return emb, wih_t, whh_t, xbias, bn


def _get_nc_and_inmaps(input, embedding, W_ih, W_hh, b_ih, b_hh, ts):
    input = np.asarray(input)
    if "nc" not in _CACHE or _CACHE.get("ts") != ts:
        _CACHE["nc"] = _build(ts)
        _CACHE["ts"] = ts
    nc = _CACHE["nc"]

    emb, wih_t, whh_t, xbias, bn = _prep_shared(embedding, W_ih, W_hh, b_ih, b_hh)

    in_maps = []
    for c in range(NCORES):
        ids = np.asarray(input[c * BL : (c + 1) * BL, :ts], np.int32)
        idx = np.ascontiguousarray(ids.T.reshape(-1).reshape(ts * BL // P, P).T)
        in_maps.append(
            {
                "emb": emb,
                "idx": idx,
                "wih_t": wih_t,
                "whh_t": whh_t,
                "xbias": xbias,
                "bn": bn,
            }
        )
    return nc, in_maps


def run_traced(input, embedding, W_ih, W_hh, b_ih, b_hh, _t_steps: int = T):
    nc, in_maps = _get_nc_and_inmaps(input, embedding, W_ih, W_hh, b_ih, b_hh, _t_steps)
    return bass_utils.run_bass_kernel_spmd(
        nc, in_maps, core_ids=list(range(NCORES)), trace=True, trace_cores=[0]
    )


def kernel(input, embedding, W_ih, W_hh, b_ih, b_hh, _t_steps: int = T):
    ts = _t_steps
    nc, in_maps = _get_nc_and_inmaps(input, embedding, W_ih, W_hh, b_ih, b_hh, ts)

    res = bass_utils.run_bass_kernel_spmd(nc, in_maps, core_ids=list(range(NCORES)))

    outputs = np.empty((ts, B, H), np.float32)
    for c in range(NCORES):
        buf = np.asarray(res.results[c]["out"], np.float32).reshape(P, ts, 2, BL)
        outputs[:, c * BL : (c + 1) * BL, :] = (
            buf.transpose(1, 3, 2, 0).reshape(ts, BL, H)
        )
    hidden = outputs[-1][None]
    return outputs, hidden


if __name__ == "__main__":
    rng = np.random.default_rng(0)
    ts = int(sys.argv[1]) if len(sys.argv) > 1 else 32
    inp = rng.integers(0, V, (B, T)).astype(np.int32)
    emb = rng.uniform(-0.05, 0.05, (V, E)).astype(np.float32)
    emb[0] = 0
    wih = rng.uniform(-0.05, 0.05, (G, E)).astype(np.float32)
    whh = rng.uniform(-0.05, 0.05, (G, H)).astype(np.float32)
    bih = rng.uniform(-0.05, 0.05, G).astype(np.float32)
    bhh = rng.uniform(-0.05, 0.05, G).astype(np.float32)

    outs, hid = kernel(inp, emb, wih, whh, bih, bhh, _t_steps=ts)

    x = emb[inp[:, :ts]].transpose(1, 0, 2)
    xp = x @ wih.T + bih
    h = np.zeros((B, H), np.float32)
    sig = lambda a: 1.0 / (1.0 + np.exp(-a))
    ref = np.empty((ts, B, H), np.float32)
    for t in range(ts):
        hp = h @ whh.T + bhh
        xr, xz, xn = np.split(xp[t], 3, -1)
        hr, hz, hn = np.split(hp, 3, -1)
        r = sig(xr + hr)
        z = sig(xz + hz)
        n = np.tanh(xn + r * hn)
        h = (1 - z) * n + z * h
        ref[t] = h
    err = np.abs(outs - ref).max() / (np.abs(ref).max() + 1e-12)
    print("max err vs absmax:", err)
    print("sample:", outs[0, 0, :4], ref[0, 0, :4])


# revision 13
# speedup vs baseline: 1.3207x; 1.0014x over previous
import sys

for _p in ("/opt/trn_rl_repo", "/root/.axon_site"):
    if _p not in sys.path:
        sys.path.insert(0, _p)

from contextlib import ExitStack

import numpy as np
import ml_dtypes

import concourse.bass as bass
import concourse.bacc as bacc
import concourse.mybir as mybir
import concourse.tile as tile
from concourse import bass_utils
from concourse.masks import make_identity

F32 = mybir.dt.float32
F32R = mybir.dt.float32r
BF16 = mybir.dt.float16
I32 = mybir.dt.int32
AF = mybir.ActivationFunctionType

V, E, H = 50000, 256, 256
B, T = 64, 512
G = 3 * H
NCORES = 8
BL = B // NCORES
P = 128
NJ = G // P
W = 2 * BL

_CACHE: dict = {}


def _build(t_steps: int = T):
    n_tok = t_steps * BL
    n_gather = n_tok // P
    chunk_tok = 512 if n_tok >= 512 else n_tok
    n_chunks = n_tok // chunk_tok
    gpc = chunk_tok // P

    nc = bacc.Bacc("TRN2", debug=False, num_devices=NCORES)

    emb = nc.dram_tensor("emb", [V, E], F32, kind="ExternalInput").ap()
    idx = nc.dram_tensor("idx", [P, n_gather], I32, kind="ExternalInput").ap()
    wih_t = nc.dram_tensor("wih_t", [E, G], BF16, kind="ExternalInput").ap()
    whh_t = nc.dram_tensor("whh_t", [E, G], BF16, kind="ExternalInput").ap()
    xbias = nc.dram_tensor("xbias", [P, NJ], F32, kind="ExternalInput").ap()
    bn = nc.dram_tensor("bn", [P, W], BF16, kind="ExternalInput").ap()
    out_d = nc.dram_tensor("out", [P, t_steps * W], BF16, kind="ExternalOutput").ap()

    with tile.TileContext(nc) as tc, ExitStack() as ctx:
        const = ctx.enter_context(tc.tile_pool(name="const", bufs=1))
        big = ctx.enter_context(tc.tile_pool(name="big", bufs=1))

        ident = const.tile([P, P], F32)
        make_identity(nc, ident[:])
        idx_sb = const.tile([P, n_gather], I32)
        nc.sync.dma_start(idx_sb[:], idx[:])
        wih_sb = [const.tile([P, G], BF16, name=f"wih{k}") for k in range(2)]
        whh_sb = [const.tile([P, G], BF16, name=f"whh{k}") for k in range(2)]
        for k in range(2):
            nc.sync.dma_start(wih_sb[k][:], wih_t[P * k : P * (k + 1), :])
            nc.sync.dma_start(whh_sb[k][:], whh_t[P * k : P * (k + 1), :])
        xbias_sb = const.tile([P, NJ], F32)
        nc.sync.dma_start(xbias_sb[:], xbias[:])
        bn_sb = const.tile([P, W], BF16)
        nc.sync.dma_start(bn_sb[:], bn[:])
        identb = const.tile([P, P], BF16)
        make_identity(nc, identb[:])
        h0 = const.tile([P, W], BF16)
        nc.vector.memset(h0[:], 0.0)

        xp_sb = big.tile([P, t_steps, 6 * BL], BF16)
        out_sb = big.tile([P, t_steps, W], BF16)

        gat = ctx.enter_context(tc.tile_pool(name="gat", bufs=3))
        embt = ctx.enter_context(tc.tile_pool(name="embt", bufs=4))
        tpps = ctx.enter_context(tc.tile_pool(name="tpps", bufs=2, space="PSUM"))
        xpps = ctx.enter_context(tc.tile_pool(name="xpps", bufs=2, space="PSUM"))

        chunks = []
        pos = 0
        while pos < n_tok:
            size = 128 if pos < 512 else 512
            size = min(size, n_tok - pos)
            chunks.append((pos, size))
            pos += size

        def chunk_thunks(ci, start, size):
            gpc = size // P
            et = [
                embt.tile([P, size], BF16, tag=f"et{k}", name=f"et{k}_{ci}")
                for k in range(2)
            ]

            def gather(g):
                def f():
                    gt = gat.tile([P, E], F32, tag="gt", name=f"gt_{ci}_{g}")
                    nc.gpsimd.indirect_dma_start(
                        out=gt[:],
                        out_offset=None,
                        in_=emb[:],
                        in_offset=bass.IndirectOffsetOnAxis(
                            ap=idx_sb[:, start // P + g : start // P + g + 1], axis=0
                        ),
                    )
                    return gt
                return f

            gt_holder = {}

            def do_gather(g):
                def f():
                    gt_holder[g] = gather(g)()
                return f

            def transpose_copy(g, k):
                def f():
                    gt = gt_holder[g]
                    tp = tpps.tile([P, P], F32, tag="tp", name=f"tp_{ci}_{g}_{k}")
                    nc.tensor.transpose(tp[:], gt[:, P * k : P * (k + 1)], ident[:])
                    dst = et[k][:, P * g : P * (g + 1)]
                    if (g * 2 + k) % 2 == 0:
                        nc.vector.tensor_copy(dst, tp[:])
                    else:
                        nc.scalar.copy(dst, tp[:])
                return f

            def gemm_repack(j):
                def f():
                    xpp = xpps.tile([P, size], F32, tag="xpp", name=f"xpp_{ci}_{j}")
                    for k in range(2):
                        nc.tensor.matmul(
                            xpp[:],
                            lhsT=wih_sb[k][:, P * j : P * (j + 1)],
                            rhs=et[k][:],
                            start=(k == 0),
                            stop=(k == 1),
                        )
                    tpc = size // BL
                    t0 = start // BL
                    dst = xp_sb[:, t0 : t0 + tpc, BL * j : BL * (j + 1)]
                    src = xpp[:].rearrange("p (t b) -> p t b", b=BL)
                    if j % 2 == 0:
                        nc.vector.tensor_scalar_add(dst, src, xbias_sb[:, j : j + 1])
                    else:
                        nc.scalar.add(dst, src, xbias_sb[:, j : j + 1])
                return f

            out = []
            for g in range(gpc):
                out.append(do_gather(g))
                for k in range(2):
                    out.append(transpose_copy(g, k))
            for j in range(NJ):
                out.append(gemm_repack(j))
            return out

        for ci, (start, size) in enumerate(chunks):
            for f in chunk_thunks(ci, start, size):
                f()

        with (
            tc.tile_pool(name="rzps", bufs=2, space="PSUM") as rzps,
            tc.tile_pool(name="nps", bufs=2, space="PSUM") as nps,
            tc.tile_pool(name="gates", bufs=3) as gp,
        ):
            def inject_const(rzp, npp, t, with_stop):
                for j in range(4):
                    nc.tensor.matmul(
                        rzp[:, BL * j : BL * (j + 1)],
                        lhsT=identb[:],
                        rhs=xp_sb[:, t, BL * j : BL * (j + 1)],
                        start=(j == 0),
                        stop=(with_stop and j == 3),
                        skip_group_check=True,
                    )
                for j in range(2):
                    nc.tensor.matmul(
                        npp[:, BL * j : BL * (j + 1)],
                        lhsT=identb[:],
                        rhs=bn_sb[:, BL * j : BL * (j + 1)],
                        start=(j == 0),
                        stop=(with_stop and j == 1),
                        skip_group_check=True,
                    )

            def w_mms(rzp, npp, cpart, with_stop):
                rhs = [cpart[:, BL * k : BL * (k + 1)] for k in range(2)]
                for j in range(4):
                    for k in range(2):
                        nc.tensor.matmul(
                            rzp[:, BL * j : BL * (j + 1)],
                            lhsT=whh_sb[k][:, P * j : P * (j + 1)],
                            rhs=rhs[k],
                            start=False,
                            stop=(with_stop and j == 3 and k == 1),
                            skip_group_check=True,
                        )
                for j in range(2):
                    for k in range(2):
                        nc.tensor.matmul(
                            npp[:, BL * j : BL * (j + 1)],
                            lhsT=whh_sb[k][:, P * (j + 4) : P * (j + 5)],
                            rhs=rhs[k],
                            start=False,
                            stop=(with_stop and j == 1 and k == 1),
                            skip_group_check=True,
                        )

            h_prev = h0
            rzp_t = rzps.tile([P, 4 * BL], F32, tag="rzp", name="rzp_0")
            npp_t = nps.tile([P, W], F32, tag="npp", name="npp_0")
            inject_const(rzp_t, npp_t, 0, with_stop=True)
            for t in range(t_steps):
                last = t + 1 >= t_steps
                if not last:
                    rzp_n = rzps.tile([P, 4 * BL], F32, tag="rzp", name=f"rzp_{t+1}")
                    npp_n = nps.tile([P, W], F32, tag="npp", name=f"npp_{t+1}")
                    inject_const(rzp_n, npp_n, t + 1, with_stop=False)
                rz = gp.tile([P, 4 * BL], F32, tag="rz")
                nc.scalar.activation(rz[:], rzp_t[:], AF.Sigmoid)
                zc = gp.tile([P, W], F32, tag="zc")
                nc.scalar.activation(zc[:], rzp_t[:, 2 * BL : 4 * BL], AF.Sigmoid, scale=-1.0)
                u = gp.tile([P, W], F32, tag="u")
                nc.vector.tensor_mul(u[:], rz[:, 0 : 2 * BL], npp_t[:])
                v = gp.tile([P, W], F32, tag="v")
                nc.vector.tensor_add(v[:], u[:], xp_sb[:, t, 4 * BL : 6 * BL])
                c1 = gp.tile([P, W], BF16, tag="c1")
                nc.vector.tensor_mul(c1[:], rz[:, 2 * BL : 4 * BL], h_prev[:])
                if not last:
                    w_mms(rzp_n, npp_n, c1, with_stop=False)
                n_ = gp.tile([P, W], F32, tag="n")
                nc.scalar.activation(n_[:], v[:], AF.Tanh)
                c3 = gp.tile([P, W], BF16, tag="c3")
                nc.vector.tensor_mul(c3[:], zc[:], n_[:])
                if not last:
                    w_mms(rzp_n, npp_n, c3, with_stop=True)
                h_new = out_sb[:, t, :]
                nc.vector.tensor_add(h_new, c3[:], c1[:])
                h_prev = h_new
                if not last:
                    rzp_t, npp_t = rzp_n, npp_n

        nc.sync.dma_start(out_d[:], out_sb[:].rearrange("p t w -> p (t w)"))

    nc.compile()
    return nc


def _prep_shared(embedding, W_ih, W_hh, b_ih, b_hh):
    emb = np.ascontiguousarray(np.asarray(embedding, np.float32))
    wih_t = np.ascontiguousarray(np.asarray(W_ih, np.float32).T.astype(np.float16))
    whh_t = np.ascontiguousarray(np.asarray(W_hh, np.float32).T.astype(np.float16))
    b_ih = np.asarray(b_ih, np.float32)
    b_hh = np.asarray(b_hh, np.float32)
    bias_x = b_ih.copy()
    bias_x[: 2 * H] += b_hh[: 2 * H]
    xbias = np.ascontiguousarray(bias_x.reshape(NJ, P).T)
    bn = np.ascontiguousarray(
        np.broadcast_to(b_hh[2 * H :].reshape(2, P).T[:, :, None], (P, 2, BL))
    ).reshape(P, W).astype(np.float16)
    return emb, wih_t, whh_t, xbias, bn


def _get_nc_and_inmaps(input, embedding, W_ih, W_hh, b_ih, b_hh, ts):
    input = np.asarray(input)
    if "nc" not in _CACHE or _CACHE.get("ts") != ts:
        _CACHE["nc"] = _build(ts)
        _CACHE["ts"] = ts
    nc = _CACHE["nc"]

    emb, wih_t, whh_t, xbias, bn = _prep_shared(embedding, W_ih, W_hh, b_ih, b_hh)

    in_maps = []
    for c in range(NCORES):
        ids = np.asarray(input[c * BL : (c + 1) * BL, :ts], np.int32)
        idx = np.ascontiguousarray(ids.T.reshape(-1).reshape(ts * BL // P, P).T)
        in_maps.append(
            {
                "emb": emb,
                "idx": idx,
                "wih_t": wih_t,
                "whh_t": whh_t,
                "xbias": xbias,
                "bn": bn,
            }
        )
    return nc, in_maps


def run_traced(input, embedding, W_ih, W_hh, b_ih, b_hh, _t_steps: int = T):
    nc, in_maps = _get_nc_and_inmaps(input, embedding, W_ih, W_hh, b_ih, b_hh, _t_steps)
    return bass_utils.run_bass_kernel_spmd(
        nc, in_maps, core_ids=list(range(NCORES)), trace=True, trace_cores=[0]
    )


def kernel(input, embedding, W_ih, W_hh, b_ih, b_hh, _t_steps: int = T):
    ts = _t_steps
    nc, in_maps = _get_nc_and_inmaps(input, embedding, W_ih, W_hh, b_ih, b_hh, ts)

    res = bass_utils.run_bass_kernel_spmd(nc, in_maps, core_ids=list(range(NCORES)))

    outputs = np.empty((ts, B, H), np.float32)
    for c in range(NCORES):
        buf = np.asarray(res.results[c]["out"], np.float32).reshape(P, ts, 2, BL)
        outputs[:, c * BL : (c + 1) * BL, :] = (
            buf.transpose(1, 3, 2, 0).reshape(ts, BL, H)
        )
    hidden = outputs[-1][None]
    return outputs, hidden


if __name__ == "__main__":
    rng = np.random.default_rng(0)
    ts = int(sys.argv[1]) if len(sys.argv) > 1 else 32
    inp = rng.integers(0, V, (B, T)).astype(np.int32)
    emb = rng.uniform(-0.05, 0.05, (V, E)).astype(np.float32)
    emb[0] = 0
    wih = rng.uniform(-0.05, 0.05, (G, E)).astype(np.float32)
    whh = rng.uniform(-0.05, 0.05, (G, H)).astype(np.float32)
    bih = rng.uniform(-0.05, 0.05, G).astype(np.float32)
    bhh = rng.uniform(-0.05, 0.05, G).astype(np.float32)

    outs, hid = kernel(inp, emb, wih, whh, bih, bhh, _t_steps=ts)

    x = emb[inp[:, :ts]].transpose(1, 0, 2)
    xp = x @ wih.T + bih
    h = np.zeros((B, H), np.float32)
    sig = lambda a: 1.0 / (1.0 + np.exp(-a))
    ref = np.empty((ts, B, H), np.float32)
    for t in range(ts):
        hp = h @ whh.T + bhh
        xr, xz, xn = np.split(xp[t], 3, -1)
        hr, hz, hn = np.split(hp, 3, -1)
        r = sig(xr + hr)
        z = sig(xz + hz)
        n = np.tanh(xn + r * hn)
        h = (1 - z) * n + z * h
        ref[t] = h
    err = np.abs(outs - ref).max() / (np.abs(ref).max() + 1e-12)
    print("max err vs absmax:", err)
    print("sample:", outs[0, 0, :4], ref[0, 0, :4])
